# revision 9
# baseline (speedup 1.0000x reference)
"""DeFeat distillation loss on 8 Trainium2 NeuronCores (Bass/Tile).

Data-parallel over the batch dim (B=8 -> 1 batch element per core).
Per core / per pyramid level:
  adapted = W @ feat_s (1x1 conv, fp32r matmuls)        [TensorE]
  d       = (feat_t - bias) - adapted                   [VectorE, fused]
  rs_tot += rowsum(d^2)                                 [ScalarE Square+accum]
  dm      = d * mask_broadcast                          [VectorE]
  rs_gt  += rowsum(dm^2)                                [ScalarE Square+accum]
Masks are rasterized on host (tiny: <90KB/core) and broadcast across
partitions on GpSimd. Per-(tile,chunk) row sums land in [128, 88] output
buffers; the host sums partials across cores/slots, applies sqrt and the
gt/bg weights.
"""

import os
import sys

for _p in ("/opt/trn_rl_repo", os.path.expanduser("~/.axon_site/_ro/trn_rl_repo")):
    if os.path.isdir(_p) and _p not in sys.path:
        sys.path.insert(0, _p)

import numpy as np

WEIGHT_GT = 0.004
WEIGHT_BG = 0.0002
STRIDES = (8, 16, 32, 64, 128)
SIZES = (128, 64, 32, 16, 8)
HWS = tuple(s * s for s in SIZES)          # (16384, 4096, 1024, 256, 64)
B, C, NBOX = 8, 256, 16
N_CORES = 8
TILE_N = 512
# per-level DMA block widths (columns of the [C, HW] feature view)
BLOCK_W = (2048, 2048, 1024, 256, 64)
N_LEVELS = 5

# tiles per level and slot bookkeeping: slot = 2*tile_idx + oc
TILES_PER_LEVEL = tuple(-(-hw // TILE_N) for hw in HWS)   # (32, 8, 2, 1, 1)
TOT_TILES = sum(TILES_PER_LEVEL)                          # 44
N_SLOTS = 2 * TOT_TILES                                   # 88
LEVEL_TILE_OFF = tuple(sum(TILES_PER_LEVEL[:i]) for i in range(N_LEVELS + 1))
MASK_LEN = sum(HWS)                                       # 21824


def _build_module():
    import concourse.mybir as mybir
    from concourse import bacc
    from concourse.tile import TileContext

    dt = mybir.dt
    nc = bacc.Bacc("TRN2", target_bir_lowering=False, debug=False,
                   num_devices=N_CORES)

    fs = [nc.dram_tensor(f"fs{l}", [C, HWS[l]], dt.float32, kind="ExternalInput")
          for l in range(N_LEVELS)]
    ft = [nc.dram_tensor(f"ft{l}", [C, HWS[l]], dt.float32, kind="ExternalInput")
          for l in range(N_LEVELS)]
    wt_d = nc.dram_tensor("wt", [128, N_LEVELS * 4 * 128], dt.float32,
                          kind="ExternalInput")
    bias_d = nc.dram_tensor("bias", [128, N_LEVELS * 2], dt.float32,
                            kind="ExternalInput")
    mask_d = nc.dram_tensor("mask", [1, MASK_LEN], dt.float32,
                            kind="ExternalInput")
    out_tot_d = nc.dram_tensor("out_tot", [128, N_SLOTS], dt.float32,
                               kind="ExternalOutput")
    out_gt_d = nc.dram_tensor("out_gt", [128, N_SLOTS], dt.float32,
                              kind="ExternalOutput")

    f32r = dt.float32r
    SUB = mybir.AluOpType.subtract
    MUL = mybir.AluOpType.mult
    SQUARE = mybir.ActivationFunctionType.Square

    with TileContext(nc) as tc:
        with (
            tc.tile_pool(name="const", bufs=1) as const_pool,
            tc.tile_pool(name="feat", bufs=2) as feat_pool,
            tc.tile_pool(name="maskp", bufs=2) as mask_pool,
            tc.tile_pool(name="work", bufs=4) as work_pool,
            tc.tile_pool(name="acc", bufs=1) as acc_pool,
            tc.tile_pool(name="ps", bufs=4, space="PSUM") as psum_pool,
        ):
            wt = const_pool.tile([128, N_LEVELS * 4 * 128], f32r)
            nc.sync.dma_start(out=wt[:], in_=wt_d[:].bitcast(f32r))
            bias = const_pool.tile([128, N_LEVELS * 2], dt.float32)
            nc.sync.dma_start(out=bias[:], in_=bias_d[:])

            rs_tot = acc_pool.tile([128, N_SLOTS], dt.float32)
            rs_gt = acc_pool.tile([128, N_SLOTS], dt.float32)

            mask_off = 0
            for lvl in range(N_LEVELS):
                hw = HWS[lvl]
                bw = BLOCK_W[lvl]
                for c0 in range(0, hw, bw):
                    w_blk = min(bw, hw - c0)
                    s_lo = feat_pool.tile([128, bw], f32r, tag="s_lo")
                    s_hi = feat_pool.tile([128, bw], f32r, tag="s_hi")
                    t_lo = feat_pool.tile([128, bw], dt.float32, tag="t_lo")
                    t_hi = feat_pool.tile([128, bw], dt.float32, tag="t_hi")
                    nc.sync.dma_start(out=s_lo[:, :w_blk],
                                      in_=fs[lvl][0:128, c0:c0 + w_blk].bitcast(f32r))
                    nc.sync.dma_start(out=s_hi[:, :w_blk],
                                      in_=fs[lvl][128:256, c0:c0 + w_blk].bitcast(f32r))
                    nc.sync.dma_start(out=t_lo[:, :w_blk],
                                      in_=ft[lvl][0:128, c0:c0 + w_blk])
                    nc.sync.dma_start(out=t_hi[:, :w_blk],
                                      in_=ft[lvl][128:256, c0:c0 + w_blk])

                    m_row = mask_pool.tile([1, bw], dt.float32, tag="m_row")
                    nc.sync.dma_start(
                        out=m_row[:, :w_blk],
                        in_=mask_d[0:1, mask_off + c0:mask_off + c0 + w_blk])
                    mb = mask_pool.tile([128, bw], dt.float32, tag="mb")
                    nc.gpsimd.partition_broadcast(mb[:, :w_blk], m_row[:, :w_blk],
                                                  channels=128)

                    t_chunks = (t_lo, t_hi)
                    for j in range(0, w_blk, TILE_N):
                        n = min(TILE_N, w_blk - j)
                        tile_idx = LEVEL_TILE_OFF[lvl] + (c0 + j) // TILE_N
                        for oc in range(2):
                            slot = 2 * tile_idx + oc
                            widx = (lvl * 2 + oc) * 2
                            ps = psum_pool.tile([128, TILE_N], dt.float32,
                                                tag="ps")
                            nc.tensor.matmul(
                                ps[:, :n],
                                wt[:, (widx + 0) * 128:(widx + 1) * 128],
                                s_lo[:, j:j + n],
                                start=True, stop=False)
                            nc.tensor.matmul(
                                ps[:, :n],
                                wt[:, (widx + 1) * 128:(widx + 2) * 128],
                                s_hi[:, j:j + n],
                                start=False, stop=True)
                            d = work_pool.tile([128, TILE_N], dt.float32, tag="d")
                            nc.vector.scalar_tensor_tensor(
                                d[:, :n],
                                t_chunks[oc][:, j:j + n],
                                bias[:, lvl * 2 + oc:lvl * 2 + oc + 1],
                                ps[:, :n],
                                op0=SUB, op1=SUB)
                            sq0 = work_pool.tile([128, TILE_N], dt.float32,
                                                 tag="sq0")
                            nc.scalar.activation(
                                sq0[:, :n], d[:, :n], SQUARE,
                                accum_out=rs_tot[:, slot:slot + 1])
                            dm = work_pool.tile([128, TILE_N], dt.float32,
                                                tag="dm")
                            nc.vector.tensor_tensor(
                                dm[:, :n], d[:, :n], mb[:, j:j + n], op=MUL)
                            sq1 = work_pool.tile([128, TILE_N], dt.float32,
                                                 tag="sq1")
                            nc.scalar.activation(
                                sq1[:, :n], dm[:, :n], SQUARE,
                                accum_out=rs_gt[:, slot:slot + 1])
                mask_off += hw

            nc.sync.dma_start(out=out_tot_d[:], in_=rs_tot[:])
            nc.sync.dma_start(out=out_gt_d[:], in_=rs_gt[:])

    nc.compile()
    return nc


def _rasterize_masks(gt_bboxes):
    """Host-side mask rasterization, mirroring reference.gt_mask in fp32.

    Returns [B, MASK_LEN] float32 (per-level masks concatenated)."""
    out = np.zeros((B, MASK_LEN), np.float32)
    for lvl in range(N_LEVELS):
        h = w = SIZES[lvl]
        stride = np.float32(STRIDES[lvl])
        off = sum(HWS[:lvl])
        q = np.floor(gt_bboxes.astype(np.float32) / stride).astype(np.int32)
        lx = np.minimum(q[..., 0], w - 1)
        ly = np.minimum(q[..., 1], h - 1)
        rx = np.minimum(q[..., 2], w - 1)
        ry = np.minimum(q[..., 3], h - 1)
        for b in range(B):
            m = np.zeros((h, w), bool)
            for i in range(gt_bboxes.shape[1]):
                if lx[b, i] == rx[b, i] or ly[b, i] == ry[b, i]:
                    m[ly[b, i], lx[b, i]] = True
                else:
                    m[ly[b, i]:ry[b, i], lx[b, i]:rx[b, i]] = True
            out[b, off:off + h * w] = m.reshape(-1).astype(np.float32)
    return out


_NC_CACHE = None


def _get_nc():
    global _NC_CACHE
    if _NC_CACHE is None:
        _NC_CACHE = _build_module()
    return _NC_CACHE


def _run(in_maps, trace=False, trace_cores=None):
    from concourse.bass_utils import run_bass_kernel_spmd

    kwargs = {}
    if trace:
        kwargs.update(trace=True, trace_cores=trace_cores or [0])
    return run_bass_kernel_spmd(_get_nc(), in_maps, core_ids=list(range(N_CORES)),
                                **kwargs)


def kernel(_trace=False, _return_results=False, **inputs):
    gt_bboxes = np.asarray(inputs["gt_bboxes"], np.float32)
    masks = _rasterize_masks(gt_bboxes)

    # replicated weights: wt[c_local, ((lvl*2+oc)*2+kc)*128 + o_local]
    #   = w_lvl[oc*128 + o_local, kc*128 + c_local]
    wt_packed = np.zeros((128, N_LEVELS * 4 * 128), np.float32)
    bias_packed = np.zeros((128, N_LEVELS * 2), np.float32)
    for lvl in range(N_LEVELS):
        w = np.asarray(inputs[f"adapt_w{lvl}"], np.float32)
        bvec = np.asarray(inputs[f"adapt_b{lvl}"], np.float32)
        for oc in range(2):
            bias_packed[:, lvl * 2 + oc] = bvec[oc * 128:(oc + 1) * 128]
            for kc in range(2):
                idx = (lvl * 2 + oc) * 2 + kc
                blk = w[oc * 128:(oc + 1) * 128, kc * 128:(kc + 1) * 128]
                wt_packed[:, idx * 128:(idx + 1) * 128] = blk.T

    in_maps = []
    for b in range(N_CORES):
        m = {"wt": wt_packed, "bias": bias_packed,
             "mask": masks[b:b + 1]}
        for lvl in range(N_LEVELS):
            m[f"fs{lvl}"] = np.ascontiguousarray(
                np.asarray(inputs[f"feat_s{lvl}"][b], np.float32).reshape(C, HWS[lvl]))
            m[f"ft{lvl}"] = np.ascontiguousarray(
                np.asarray(inputs[f"feat_t{lvl}"][b], np.float32).reshape(C, HWS[lvl]))
        in_maps.append(m)

    res = _run(in_maps, trace=_trace)

    loss = np.float64(0.0)
    for lvl in range(N_LEVELS):
        lo, hi = 2 * LEVEL_TILE_OFF[lvl], 2 * LEVEL_TILE_OFF[lvl + 1]
        s_tot = np.float64(0.0)
        s_gt = np.float64(0.0)
        for c in range(N_CORES):
            s_tot += res.results[c]["out_tot"][:, lo:hi].astype(np.float64).sum()
            s_gt += res.results[c]["out_gt"][:, lo:hi].astype(np.float64).sum()
        s_bg = s_tot - s_gt
        loss += WEIGHT_GT * np.sqrt(s_gt + 1e-8) + WEIGHT_BG * np.sqrt(s_bg + 1e-8)

    out = np.array(loss, dtype=np.float32)
    if _return_results:
        return out, res
    return out


# revision 10
# speedup vs baseline: 1.1259x; 1.1259x over previous
"""DeFeat distillation loss on 8 Trainium2 NeuronCores (Bass/Tile).

Data-parallel over the batch dim (B=8 -> 1 batch element per core).
Per core / per pyramid level (features viewed as [C=256, H*W]):
  adapted = W @ feat_s (1x1 conv, fp32r matmuls)             [TensorE]
  d       = (feat_t - bias) - adapted                        [VectorE, fused]
  rs_tot[slot] = rowsum(d^2), dd = d^2 (bf16)                [ScalarE Square+accum]
  rs_gt[slot]  = rowsum(dd * mask)                           [VectorE, fused accum]
Masks are rasterized on host (<90KB/core, bf16) and broadcast across
partitions on GpSimd. Per-(block,chunk) row sums land in [128, n_slots]
output buffers; the host sums partials across cores/slots, applies sqrt
and the gt/bg weights.
"""

import os
import sys

for _p in ("/opt/trn_rl_repo", os.path.expanduser("~/.axon_site/_ro/trn_rl_repo")):
    if os.path.isdir(_p) and _p not in sys.path:
        sys.path.insert(0, _p)

import numpy as np

WEIGHT_GT = 0.004
WEIGHT_BG = 0.0002
STRIDES = (8, 16, 32, 64, 128)
SIZES = (128, 64, 32, 16, 8)
HWS = tuple(s * s for s in SIZES)          # (16384, 4096, 1024, 256, 64)
B, C, NBOX = 8, 256, 16
N_CORES = 8
TILE_N = 512
BLOCK_W = (2048, 2048, 1024, 256, 64)      # per-level DMA block width
N_LEVELS = 5
LEVEL_ORDER = (4, 3, 2, 1, 0)              # small levels first (DMA ramp)
MASK_LEN = sum(HWS)                        # 21824
MASK_OFF = tuple(sum(HWS[:i]) for i in range(N_LEVELS))

# block list in processing order; slots are 2 per block (one per oc chunk)
BLOCKS = []
for _l in LEVEL_ORDER:
    for _c0 in range(0, HWS[_l], BLOCK_W[_l]):
        BLOCKS.append((_l, _c0, min(BLOCK_W[_l], HWS[_l] - _c0)))
N_SLOTS = 2 * len(BLOCKS)                  # 26


def _build_module():
    import concourse.mybir as mybir
    from concourse import bacc
    from concourse.tile import TileContext

    dt = mybir.dt
    nc = bacc.Bacc("TRN2", target_bir_lowering=False, debug=False,
                   num_devices=N_CORES)

    fs = [nc.dram_tensor(f"fs{l}", [C, HWS[l]], dt.float32, kind="ExternalInput")
          for l in range(N_LEVELS)]
    ft = [nc.dram_tensor(f"ft{l}", [C, HWS[l]], dt.float32, kind="ExternalInput")
          for l in range(N_LEVELS)]
    wt_d = nc.dram_tensor("wt", [128, N_LEVELS * 4 * 128], dt.float32,
                          kind="ExternalInput")
    bias_d = nc.dram_tensor("bias", [128, N_LEVELS * 2], dt.float32,
                            kind="ExternalInput")
    mask_d = nc.dram_tensor("mask", [1, MASK_LEN], dt.bfloat16,
                            kind="ExternalInput")
    out_tot_d = nc.dram_tensor("out_tot", [128, N_SLOTS], dt.float32,
                               kind="ExternalOutput")
    out_gt_d = nc.dram_tensor("out_gt", [128, N_SLOTS], dt.float32,
                              kind="ExternalOutput")

    f32r = dt.float32r
    SUB = mybir.AluOpType.subtract
    MUL = mybir.AluOpType.mult
    SQUARE = mybir.ActivationFunctionType.Square

    with TileContext(nc) as tc:
        with (
            tc.tile_pool(name="const", bufs=1) as const_pool,
            tc.tile_pool(name="feat", bufs=2) as feat_pool,
            tc.tile_pool(name="maskp", bufs=2) as mask_pool,
            tc.tile_pool(name="work", bufs=2) as work_pool,
            tc.tile_pool(name="acc", bufs=1) as acc_pool,
            tc.tile_pool(name="ps", bufs=4, space="PSUM") as psum_pool,
        ):
            wt = const_pool.tile([128, N_LEVELS * 4 * 128], f32r)
            nc.sync.dma_start(out=wt[:], in_=wt_d[:].bitcast(f32r))
            bias = const_pool.tile([128, N_LEVELS * 2], dt.float32)
            nc.sync.dma_start(out=bias[:], in_=bias_d[:])

            rs_tot = acc_pool.tile([128, N_SLOTS], dt.float32)
            rs_gt = acc_pool.tile([128, N_SLOTS], dt.float32)

            for bi, (lvl, c0, w_blk) in enumerate(BLOCKS):
                bw = BLOCK_W[lvl]
                s_lo = feat_pool.tile([128, bw], f32r, tag="s_lo")
                s_hi = feat_pool.tile([128, bw], f32r, tag="s_hi")
                t_lo = feat_pool.tile([128, bw], dt.float32, tag="t_lo")
                t_hi = feat_pool.tile([128, bw], dt.float32, tag="t_hi")
                nc.sync.dma_start(out=s_lo[:, :w_blk],
                                  in_=fs[lvl][0:128, c0:c0 + w_blk].bitcast(f32r))
                nc.sync.dma_start(out=s_hi[:, :w_blk],
                                  in_=fs[lvl][128:256, c0:c0 + w_blk].bitcast(f32r))
                nc.sync.dma_start(out=t_lo[:, :w_blk],
                                  in_=ft[lvl][0:128, c0:c0 + w_blk])
                nc.sync.dma_start(out=t_hi[:, :w_blk],
                                  in_=ft[lvl][128:256, c0:c0 + w_blk])

                moff = MASK_OFF[lvl] + c0
                m_row = mask_pool.tile([1, bw], dt.bfloat16, tag="m_row")
                nc.sync.dma_start(out=m_row[:, :w_blk],
                                  in_=mask_d[0:1, moff:moff + w_blk])
                mb = mask_pool.tile([128, bw], dt.bfloat16, tag="mb")
                nc.gpsimd.partition_broadcast(mb[:, :w_blk], m_row[:, :w_blk],
                                              channels=128)

                t_chunks = (t_lo, t_hi)
                for oc in range(2):
                    slot = 2 * bi + oc
                    widx = (lvl * 2 + oc) * 2
                    d_blk = work_pool.tile([128, bw], dt.float32, tag="d")
                    for j in range(0, w_blk, TILE_N):
                        n = min(TILE_N, w_blk - j)
                        ps = psum_pool.tile([128, TILE_N], dt.float32, tag="ps")
                        nc.tensor.matmul(
                            ps[:, :n],
                            wt[:, (widx + 0) * 128:(widx + 1) * 128],
                            s_lo[:, j:j + n],
                            start=True, stop=False)
                        nc.tensor.matmul(
                            ps[:, :n],
                            wt[:, (widx + 1) * 128:(widx + 2) * 128],
                            s_hi[:, j:j + n],
                            start=False, stop=True)
                        nc.vector.scalar_tensor_tensor(
                            d_blk[:, j:j + n],
                            t_chunks[oc][:, j:j + n],
                            bias[:, lvl * 2 + oc:lvl * 2 + oc + 1],
                            ps[:, :n],
                            op0=SUB, op1=SUB)
                    dd_blk = work_pool.tile([128, bw], dt.bfloat16, tag="dd")
                    nc.scalar.activation(
                        dd_blk[:, :w_blk], d_blk[:, :w_blk], SQUARE,
                        accum_out=rs_tot[:, slot:slot + 1])
                    scr_blk = work_pool.tile([128, bw], dt.bfloat16, tag="scr")
                    nc.vector.scalar_tensor_tensor(
                        scr_blk[:, :w_blk],
                        dd_blk[:, :w_blk],
                        1.0,
                        mb[:, :w_blk],
                        op0=MUL, op1=MUL,
                        accum_out=rs_gt[:, slot:slot + 1])

            nc.sync.dma_start(out=out_tot_d[:], in_=rs_tot[:])
            nc.sync.dma_start(out=out_gt_d[:], in_=rs_gt[:])

    nc.compile()
    return nc


def _rasterize_masks(gt_bboxes):
    """Host-side mask rasterization, mirroring reference.gt_mask in fp32.

    Returns [B, MASK_LEN] float32 (per-level masks concatenated)."""
    out = np.zeros((B, MASK_LEN), np.float32)
    for lvl in range(N_LEVELS):
        h = w = SIZES[lvl]
        stride = np.float32(STRIDES[lvl])
        off = MASK_OFF[lvl]
        q = np.floor(gt_bboxes.astype(np.float32) / stride).astype(np.int32)
        lx = np.minimum(q[..., 0], w - 1)
        ly = np.minimum(q[..., 1], h - 1)
        rx = np.minimum(q[..., 2], w - 1)
        ry = np.minimum(q[..., 3], h - 1)
        for b in range(B):
            m = np.zeros((h, w), bool)
            for i in range(gt_bboxes.shape[1]):
                if lx[b, i] == rx[b, i] or ly[b, i] == ry[b, i]:
                    m[ly[b, i], lx[b, i]] = True
                else:
                    m[ly[b, i]:ry[b, i], lx[b, i]:rx[b, i]] = True
            out[b, off:off + h * w] = m.reshape(-1).astype(np.float32)
    return out


_NC_CACHE = None


def _get_nc():
    global _NC_CACHE
    if _NC_CACHE is None:
        _NC_CACHE = _build_module()
    return _NC_CACHE


def _run(in_maps, trace=False, trace_cores=None):
    from concourse.bass_utils import run_bass_kernel_spmd

    kwargs = {}
    if trace:
        kwargs.update(trace=True, trace_cores=trace_cores or [0])
    return run_bass_kernel_spmd(_get_nc(), in_maps, core_ids=list(range(N_CORES)),
                                **kwargs)


def kernel(_trace=False, _return_results=False, **inputs):
    import ml_dtypes

    gt_bboxes = np.asarray(inputs["gt_bboxes"], np.float32)
    masks = _rasterize_masks(gt_bboxes).astype(ml_dtypes.bfloat16)

    # replicated weights: wt[c_local, ((lvl*2+oc)*2+kc)*128 + o_local]
    #   = w_lvl[oc*128 + o_local, kc*128 + c_local]
    wt_packed = np.zeros((128, N_LEVELS * 4 * 128), np.float32)
    bias_packed = np.zeros((128, N_LEVELS * 2), np.float32)
    for lvl in range(N_LEVELS):
        w = np.asarray(inputs[f"adapt_w{lvl}"], np.float32)
        bvec = np.asarray(inputs[f"adapt_b{lvl}"], np.float32)
        for oc in range(2):
            bias_packed[:, lvl * 2 + oc] = bvec[oc * 128:(oc + 1) * 128]
            for kc in range(2):
                idx = (lvl * 2 + oc) * 2 + kc
                blk = w[oc * 128:(oc + 1) * 128, kc * 128:(kc + 1) * 128]
                wt_packed[:, idx * 128:(idx + 1) * 128] = blk.T

    in_maps = []
    for b in range(N_CORES):
        m = {"wt": wt_packed, "bias": bias_packed,
             "mask": masks[b:b + 1]}
        for lvl in range(N_LEVELS):
            m[f"fs{lvl}"] = np.ascontiguousarray(
                np.asarray(inputs[f"feat_s{lvl}"][b], np.float32).reshape(C, HWS[lvl]))
            m[f"ft{lvl}"] = np.ascontiguousarray(
                np.asarray(inputs[f"feat_t{lvl}"][b], np.float32).reshape(C, HWS[lvl]))
        in_maps.append(m)

    res = _run(in_maps, trace=_trace)

    # slot -> level mapping from BLOCKS
    loss = np.float64(0.0)
    for lvl in range(N_LEVELS):
        slots = [2 * bi + oc for bi, (l, _, _) in enumerate(BLOCKS) if l == lvl
                 for oc in range(2)]
        s_tot = np.float64(0.0)
        s_gt = np.float64(0.0)
        for c in range(N_CORES):
            s_tot += res.results[c]["out_tot"][:, slots].astype(np.float64).sum()
            s_gt += res.results[c]["out_gt"][:, slots].astype(np.float64).sum()
        s_bg = s_tot - s_gt
        loss += WEIGHT_GT * np.sqrt(s_gt + 1e-8) + WEIGHT_BG * np.sqrt(s_bg + 1e-8)

    out = np.array(loss, dtype=np.float32)
    if _return_results:
        return out, res
    return out
